# revision 10
# baseline (speedup 1.0000x reference)
"""PiCO-style fused kernel for 8 Trainium2 NeuronCores (Bass/Tile).

Strategy (hardcoded from the sharding hint):
  - Data-parallel over the batch dim (4096 -> 512 rows/core) for the two
    softmaxes, the cluster matmul, the argmax pseudo-labels, and the
    per-core partial prototype scatter.
  - Prototypes replicated; the order-dependent scatter-EMA is decomposed
    per core (local suffix ranks via a triangular matmul, exact in bf16);
    the tiny [8,1000]+[8,1000,128] cross-core segment combine runs on
    the host.
  - Queue sharded by row (8192 rows/core), copied device-side
    (DRAM->DRAM); host only re-assembles full outputs from shards.
"""

import numpy as np

B, D, C, Q = 4096, 128, 1000, 65536
N_CORES = 8
BL = B // N_CORES          # 512 batch rows / core
QB = Q // N_CORES          # 8192 queue rows / core
T = BL // 128              # 4 partition tiles / core
M_EMA = 0.99
LN_M = float(np.log(0.99))
CHUNKS = ((0, 512), (512, C))   # psum-bank-sized column chunks of C

_CACHE: dict = {}


def _build():
    import concourse.bacc as bacc
    import concourse.tile as tile
    import concourse.mybir as mybir

    f32 = mybir.dt.float32
    bf16 = mybir.dt.bfloat16
    i32 = mybir.dt.int32
    u32 = mybir.dt.uint32
    Alu = mybir.AluOpType
    Act = mybir.ActivationFunctionType

    nc = bacc.Bacc("TRN2", target_bir_lowering=False, debug=False,
                   num_devices=N_CORES)

    # ---- external I/O (per-core shapes) ----
    cls_h = nc.dram_tensor("cls", [BL, C], f32, kind="ExternalInput")
    plab_h = nc.dram_tensor("plab", [BL, C], f32, kind="ExternalInput")
    q_h = nc.dram_tensor("q", [BL, D], f32, kind="ExternalInput")
    qT_h = nc.dram_tensor("qT", [D, BL], f32, kind="ExternalInput")
    k_h = nc.dram_tensor("k", [BL, D], f32, kind="ExternalInput")
    protoT_h = nc.dram_tensor("protoT", [D, C], f32, kind="ExternalInput")
    queue_h = nc.dram_tensor("queue_blk", [QB, D], f32, kind="ExternalInput")
    qp_h = nc.dram_tensor("qp_blk", [QB], f32, kind="ExternalInput")
    ones_h = nc.dram_tensor("ones128", [128, 128], bf16, kind="ExternalInput")
    tri_h = nc.dram_tensor("tri128", [128, 128], bf16, kind="ExternalInput")

    clso_h = nc.dram_tensor("cls_out", [BL, C], f32, kind="ExternalOutput")
    cluo_h = nc.dram_tensor("clu_out", [BL, C], f32, kind="ExternalOutput")
    lab_h = nc.dram_tensor("labels", [BL, 1], f32, kind="ExternalOutput")
    cnt_h = nc.dram_tensor("counts", [1, C], f32, kind="ExternalOutput")
    srT_h = nc.dram_tensor("s_rT", [D, C], f32, kind="ExternalOutput")
    qc_h = nc.dram_tensor("q_copy", [BL, D], f32, kind="ExternalOutput")
    kc_h = nc.dram_tensor("k_copy", [BL, D], f32, kind="ExternalOutput")
    quc_h = nc.dram_tensor("queue_copy", [QB, D], f32, kind="ExternalOutput")
    qpc_h = nc.dram_tensor("qp_copy", [QB], f32, kind="ExternalOutput")

    with tile.TileContext(nc) as tc:
        from concourse.tile import add_dep_helper
        with (
            tc.tile_pool(name="const", bufs=1) as constp,
            tc.tile_pool(name="io", bufs=4) as iop,
            tc.tile_pool(name="work", bufs=3) as workp,
            tc.tile_pool(name="oh", bufs=4) as ohp,
            tc.tile_pool(name="cols", bufs=8) as colp,
        ):
            # iota first: sole gpsimd dependency of the onehot chain
            iota_i = constp.tile([128, C], i32, tag="iota_i")
            nc.gpsimd.iota(iota_i[:], pattern=[[1, C]], base=0,
                           channel_multiplier=0)
            iota_f = constp.tile([128, C], f32, tag="iota_f")
            nc.vector.tensor_copy(iota_f[:], iota_i[:])

            # hoist every SBUF load ahead of all stores/bulk copies so the
            # latency-critical reads get HBM first; qT/protoT lead so the
            # cluster matmuls can start before the classifier loads land
            qT = constp.tile([D, BL], f32, tag="qT")
            nc.sync.dma_start(out=qT[:], in_=qT_h[:])
            protoT = constp.tile([D, C], f32, tag="protoT")
            nc.sync.dma_start(out=protoT[:], in_=protoT_h[:])
            cls_ts, plab_ts = [], []
            for t in range(T):
                rs = slice(t * 128, (t + 1) * 128)
                cls_t = iop.tile([128, C], f32, tag="cls_t", name=f"cls_t{t}")
                nc.sync.dma_start(out=cls_t[:], in_=cls_h[rs, :])
                plab_t = iop.tile([128, C], f32, tag="plab_t",
                                  name=f"plab_t{t}")
                nc.sync.dma_start(out=plab_t[:], in_=plab_h[rs, :])
                cls_ts.append(cls_t)
                plab_ts.append(plab_t)
            ones = constp.tile([128, 128], bf16, tag="ones")
            nc.sync.dma_start(out=ones[:], in_=ones_h[:])
            tri = constp.tile([128, 128], bf16, tag="tri")
            nc.sync.dma_start(out=tri[:], in_=tri_h[:])
            q_nat = []
            last_load = None
            for t in range(T):
                qn = constp.tile([128, D], f32, tag="q_nat", bufs=T,
                                 name=f"q_nat{t}")
                last_load = nc.sync.dma_start(
                    out=qn[:], in_=q_h[t * 128:(t + 1) * 128, :])
                q_nat.append(qn)

            onehot = []
            with tc.tile_pool(name="pclu", bufs=2, space="PSUM") as pclu:
                for t in range(T):
                    rs = slice(t * 128, (t + 1) * 128)
                    cls_t, plab_t = cls_ts[t], plab_ts[t]

                    # cluster logits (only needs qT/protoT -> starts early)
                    clu_ps = []
                    for ci, (c0, c1) in enumerate(CHUNKS):
                        pl = pclu.tile([128, c1 - c0], f32, tag=f"clu{ci}")
                        nc.tensor.matmul(pl[:], lhsT=qT[:, rs],
                                         rhs=protoT[:, c0:c1],
                                         start=True, stop=True)
                        clu_ps.append(pl)

                    # classifier softmax (no max-subtract: logits are O(5))
                    e_cls = workp.tile([128, C], f32, tag="e_cls")
                    sum_c = colp.tile([128, 1], f32, tag="sum_c")
                    nc.scalar.activation(e_cls[:], cls_t[:], Act.Exp,
                                         bias=0.0, scale=1.0,
                                         accum_out=sum_c[:])
                    # cluster exps right behind on ACT so a stalled output
                    # copy can never wedge them (ACT executes in order)
                    e_clu = workp.tile([128, C], f32, tag="e_clu")
                    s0 = colp.tile([128, 1], f32, tag="s0")
                    s1 = colp.tile([128, 1], f32, tag="s1")
                    for ci, (c0, c1) in enumerate(CHUNKS):
                        nc.scalar.activation(e_clu[:, c0:c1], clu_ps[ci][:],
                                             Act.Exp, bias=0.0, scale=1.0,
                                             accum_out=(s0 if ci == 0 else s1)[:])

                    # argmax path: recip cancels inside a row; skip it here
                    masked = workp.tile([128, C], f32, tag="masked")
                    nc.vector.tensor_tensor(masked[:], e_cls[:], plab_t[:],
                                            op=Alu.mult)
                    mx8 = colp.tile([128, 8], f32, tag="mx8")
                    nc.vector.max(mx8[:], masked[:])
                    ix8 = colp.tile([128, 8], u32, tag="ix8")
                    nc.vector.max_index(ix8[:], mx8[:], masked[:])
                    lab_f = colp.tile([128, 1], f32, tag="lab_f", bufs=T,
                                      name=f"lab_f{t}")
                    nc.vector.tensor_copy(lab_f[:], ix8[:, 0:1])
                    nc.sync.dma_start(out=lab_h[rs, :], in_=lab_f[:])
                    oh = ohp.tile([128, C], bf16, tag="onehot", bufs=T,
                                  name=f"onehot{t}")
                    nc.vector.tensor_scalar(oh[:], iota_f[:], lab_f[:], None,
                                            op0=Alu.is_equal)
                    onehot.append(oh)

                    # softmax outputs (leaves); per-tile slots so a slow
                    # store drain can't WAR-block the next tile
                    rec_c = colp.tile([128, 1], f32, tag="rec_c")
                    nc.vector.reciprocal(rec_c[:], sum_c[:])
                    cls_o = workp.tile([128, C], f32, tag="cls_o", bufs=T,
                                       name=f"cls_o{t}")
                    nc.scalar.activation(cls_o[:], e_cls[:], Act.Copy,
                                         bias=0.0, scale=rec_c[:])
                    nc.sync.dma_start(out=clso_h[rs, :], in_=cls_o[:])
                    ssum = colp.tile([128, 1], f32, tag="ssum")
                    nc.vector.tensor_tensor(ssum[:], s0[:], s1[:], op=Alu.add)
                    rec_u = colp.tile([128, 1], f32, tag="rec_u")
                    nc.vector.reciprocal(rec_u[:], ssum[:])
                    clu_o = workp.tile([128, C], f32, tag="clu_o", bufs=T,
                                       name=f"clu_o{t}")
                    nc.scalar.activation(clu_o[:], e_clu[:], Act.Copy,
                                         bias=0.0, scale=rec_u[:])
                    nc.sync.dma_start(out=cluo_h[rs, :], in_=clu_o[:])

                    # one bulk DRAM->DRAM queue chunk per tile iteration;
                    # gated behind the hoisted loads so the copies don't
                    # starve the latency-critical reads of HBM bandwidth
                    rows = QB // 4
                    d2d = nc.gpsimd.dma_start(
                        out=quc_h[t * rows:(t + 1) * rows, :],
                        in_=queue_h[t * rows:(t + 1) * rows, :])
                    add_dep_helper(d2d.ins, last_load.ins, sync=True,
                                   reason="bulk copy after critical loads")

            # remaining bulk passthrough copies
            for dst, src in ((qc_h, q_h), (kc_h, k_h), (qpc_h, qp_h)):
                d2d = nc.gpsimd.dma_start(out=dst[:], in_=src[:])
                add_dep_helper(d2d.ins, last_load.ins, sync=True,
                               reason="bulk copy after critical loads")

            # ---------- suffix ranks + EMA weights + scatter ----------
            ow = []
            with (
                tc.tile_pool(name="pss", bufs=2, space="PSUM") as pss,
                tc.tile_pool(name="psc", bufs=1, space="PSUM") as psc,
            ):
                for t in range(T):
                    sA = colp.tile([128, 1], f32, tag="sA")
                    sB = colp.tile([128, 1], f32, tag="sB")
                    ss_ps = []
                    for ci, (c0, c1) in enumerate(CHUNKS):
                        ps = pss.tile([128, c1 - c0], f32, tag=f"ss{ci}")
                        for tj in range(t, T):
                            nc.tensor.matmul(ps[:],
                                             lhsT=(tri if tj == t else ones)[:],
                                             rhs=onehot[tj][:, c0:c1],
                                             start=(tj == t), stop=(tj == T - 1))
                        scr = workp.tile([128, c1 - c0], f32, tag=f"ttr{ci}")
                        nc.vector.tensor_tensor(scr[:], ps[:],
                                                onehot[t][:, c0:c1],
                                                op=Alu.mult)
                        nc.vector.tensor_reduce((sA if ci == 0 else sB)[:],
                                                scr[:],
                                                axis=mybir.AxisListType.X,
                                                op=Alu.add)
                        ss_ps.append(ps)

                    if t == 0:
                        # per-class counts = SS[0,:] + onehot[0,:]
                        cnt_sb = constp.tile([1, C], f32, tag="cnt_sb")
                        for ci, (c0, c1) in enumerate(CHUNKS):
                            nc.vector.tensor_tensor(cnt_sb[0:1, c0:c1],
                                                    ss_ps[ci][0:1, :],
                                                    onehot[0][0:1, c0:c1],
                                                    op=Alu.add)
                        nc.sync.dma_start(out=cnt_h[:], in_=cnt_sb[:])

                    s_col = colp.tile([128, 1], f32, tag="s_col")
                    nc.vector.tensor_tensor(s_col[:], sA[:], sB[:], op=Alu.add)
                    wloc = colp.tile([128, 1], f32, tag="wloc", bufs=T,
                                     name=f"wloc{t}")
                    nc.scalar.activation(wloc[:], s_col[:], Act.Exp,
                                         bias=0.0, scale=LN_M)
                    o_w = ohp.tile([128, C], f32, tag="ow", bufs=T,
                                   name=f"ow{t}")
                    nc.scalar.activation(o_w[:], onehot[t][:], Act.Copy,
                                         bias=0.0, scale=wloc[:])
                    ow.append(o_w)

                # partial scatter, transposed: S_r^T = sum_t q[t]^T @ ow[t]
                for ci, (c0, c1) in enumerate(CHUNKS):
                    ps = psc.tile([128, c1 - c0], f32, tag=f"sc{ci}")
                    for t in range(T):
                        nc.tensor.matmul(ps[:], lhsT=q_nat[t][:],
                                         rhs=ow[t][:, c0:c1],
                                         start=(t == 0), stop=(t == T - 1))
                    srT_sb = workp.tile([128, c1 - c0], f32, tag=f"srT{ci}")
                    nc.scalar.copy(srT_sb[:], ps[:])
                    nc.sync.dma_start(out=srT_h[:, c0:c1], in_=srT_sb[:])

    nc.compile()
    return nc


def _get_nc():
    if "nc" not in _CACHE:
        _CACHE["nc"] = _build()
    return _CACHE["nc"]


def kernel(q, k, classfy_logits, plabel, prototypes, queue, queue_pseudo, ptr):
    import ml_dtypes
    from concourse.bass_utils import run_bass_kernel_spmd

    q = np.ascontiguousarray(np.asarray(q, np.float32))
    k = np.ascontiguousarray(np.asarray(k, np.float32))
    cls = np.ascontiguousarray(np.asarray(classfy_logits, np.float32))
    plab = np.ascontiguousarray(np.asarray(plabel, np.float32))
    proto = np.ascontiguousarray(np.asarray(prototypes, np.float32))
    queue = np.ascontiguousarray(np.asarray(queue, np.float32))
    qp = np.ascontiguousarray(np.asarray(queue_pseudo, np.float32))
    ptr = int(np.asarray(ptr))

    protoT = np.ascontiguousarray(proto.T)
    ones128 = np.ones((128, 128), ml_dtypes.bfloat16)
    tri128 = np.tril(np.ones((128, 128), np.float32), -1).astype(ml_dtypes.bfloat16)

    in_maps = []
    for r in range(N_CORES):
        bs = slice(r * BL, (r + 1) * BL)
        qs = slice(r * QB, (r + 1) * QB)
        in_maps.append({
            "cls": cls[bs], "plab": plab[bs],
            "q": q[bs], "qT": np.ascontiguousarray(q[bs].T), "k": k[bs],
            "protoT": protoT, "queue_blk": queue[qs], "qp_blk": qp[qs],
            "ones128": ones128, "tri128": tri128,
        })

    nc = _get_nc()
    res = run_bass_kernel_spmd(nc, in_maps, list(range(N_CORES)))
    _CACHE["last_result"] = res
    out = res.results

    # ---------- host: gather / combine ----------
    m = np.float32(M_EMA)
    one_minus_m = np.float32(1.0 - M_EMA)

    cls_out = np.concatenate([out[r]["cls_out"] for r in range(N_CORES)], 0)
    clu_out = np.concatenate([out[r]["clu_out"] for r in range(N_CORES)], 0)
    labels = np.concatenate([out[r]["labels"][:, 0] for r in range(N_CORES)], 0)
    q_copies = [out[r]["q_copy"] for r in range(N_CORES)]
    k_copies = [out[r]["k_copy"] for r in range(N_CORES)]
    qu_copies = [out[r]["queue_copy"] for r in range(N_CORES)]
    qp_copies = [out[r]["qp_copy"] for r in range(N_CORES)]

    cont_features = np.concatenate(q_copies + k_copies + qu_copies, 0)
    qp_dev = np.concatenate(qp_copies, 0)
    cont_labels = np.concatenate([labels, labels, qp_dev], 0)

    p0 = min(max(ptr, 0), Q - B)   # jax dynamic_update_slice clamp
    new_queue = np.concatenate(qu_copies, 0)
    new_queue[p0:p0 + B] = np.concatenate(k_copies, 0)
    new_queue_pseudo = qp_dev.copy()
    new_queue_pseudo[p0:p0 + B] = labels

    # prototype scatter-EMA segment combine (tiny)
    counts = np.stack([out[r]["counts"][0] for r in range(N_CORES)], 0)  # [8,C]
    S_all = np.stack([np.ascontiguousarray(out[r]["s_rT"].T)
                      for r in range(N_CORES)], 0)                        # [8,C,D]
    suf = np.zeros((N_CORES, C), np.float32)
    acc = np.zeros((C,), np.float32)
    for r in range(N_CORES - 1, -1, -1):
        suf[r] = acc
        acc = acc + counts[r]
    scale = np.power(m, suf).astype(np.float32)
    scatter = np.einsum("rc,rcd->cd", scale, S_all).astype(np.float32)
    decay = np.power(m, counts.sum(0)).astype(np.float32)
    pre = proto * decay[:, None] + one_minus_m * scatter
    nrm = np.sqrt((pre * pre).sum(1, keepdims=True))
    new_prototypes = (pre / np.maximum(nrm, np.float32(1e-12))).astype(np.float32)

    new_ptr = (ptr + B) % Q
    return (cls_out, clu_out, cont_features, cont_labels,
            new_prototypes, new_queue, new_queue_pseudo, new_ptr)


# revision 11
# speedup vs baseline: 1.4137x; 1.4137x over previous
"""PiCO-style fused kernel for 8 Trainium2 NeuronCores (Bass/Tile).

Strategy (hardcoded from the sharding hint):
  - Data-parallel over the batch dim (4096 -> 512 rows/core) for the two
    softmaxes, the cluster matmul, the argmax pseudo-labels, and the
    per-core partial prototype scatter.
  - Prototypes replicated; the order-dependent scatter-EMA is decomposed
    per core (local suffix ranks via a triangular matmul, exact in bf16);
    the tiny [8,1000]+[8,1000,128] cross-core segment combine runs on
    the host.
  - Queue sharded by row (8192 rows/core), copied device-side
    (DRAM->DRAM); host only re-assembles full outputs from shards.
  - Batch-tile loads/stores are fused into single wide-tile DMAs (the
    HWDGE ring is FIFO; per-DMA fixed cost dominates at 0.5MB).
"""

import numpy as np

B, D, C, Q = 4096, 128, 1000, 65536
N_CORES = 8
BL = B // N_CORES          # 512 batch rows / core
QB = Q // N_CORES          # 8192 queue rows / core
T = BL // 128              # 4 partition tiles / core
M_EMA = 0.99
LN_M = float(np.log(0.99))
CHUNKS = ((0, 512), (512, C))   # psum-bank-sized column chunks of C

_CACHE: dict = {}


def _build():
    import concourse.bacc as bacc
    import concourse.tile as tile
    import concourse.mybir as mybir
    from concourse.tile import add_dep_helper

    f32 = mybir.dt.float32
    bf16 = mybir.dt.bfloat16
    i32 = mybir.dt.int32
    u32 = mybir.dt.uint32
    Alu = mybir.AluOpType
    Act = mybir.ActivationFunctionType

    nc = bacc.Bacc("TRN2", target_bir_lowering=False, debug=False,
                   num_devices=N_CORES)

    # ---- external I/O (per-core shapes) ----
    cls_h = nc.dram_tensor("cls", [BL, C], f32, kind="ExternalInput")
    plab_h = nc.dram_tensor("plab", [BL, C], f32, kind="ExternalInput")
    q_h = nc.dram_tensor("q", [BL, D], f32, kind="ExternalInput")
    qT_h = nc.dram_tensor("qT", [D, BL], f32, kind="ExternalInput")
    k_h = nc.dram_tensor("k", [BL, D], f32, kind="ExternalInput")
    protoT_h = nc.dram_tensor("protoT", [D, C], f32, kind="ExternalInput")
    queue_h = nc.dram_tensor("queue_blk", [QB, D], f32, kind="ExternalInput")
    qp_h = nc.dram_tensor("qp_blk", [QB], f32, kind="ExternalInput")
    ones_h = nc.dram_tensor("ones128", [128, 128], bf16, kind="ExternalInput")
    tri_h = nc.dram_tensor("tri128", [128, 128], bf16, kind="ExternalInput")

    clso_h = nc.dram_tensor("cls_out", [BL, C], f32, kind="ExternalOutput")
    cluo_h = nc.dram_tensor("clu_out", [BL, C], f32, kind="ExternalOutput")
    lab_h = nc.dram_tensor("labels", [BL, 1], f32, kind="ExternalOutput")
    cnt_h = nc.dram_tensor("counts", [1, C], f32, kind="ExternalOutput")
    srT_h = nc.dram_tensor("s_rT", [D, C], f32, kind="ExternalOutput")
    qc_h = nc.dram_tensor("q_copy", [BL, D], f32, kind="ExternalOutput")
    kc_h = nc.dram_tensor("k_copy", [BL, D], f32, kind="ExternalOutput")
    quc_h = nc.dram_tensor("queue_copy", [QB, D], f32, kind="ExternalOutput")
    qpc_h = nc.dram_tensor("qp_copy", [QB], f32, kind="ExternalOutput")

    with tile.TileContext(nc) as tc:
        with (
            tc.tile_pool(name="const", bufs=1) as constp,
            tc.tile_pool(name="work", bufs=3) as workp,
            tc.tile_pool(name="oh", bufs=4) as ohp,
            tc.tile_pool(name="cols", bufs=8) as colp,
        ):
            # iota first: sole gpsimd dependency of the onehot chain
            iota_i = constp.tile([128, C], i32, tag="iota_i")
            nc.gpsimd.iota(iota_i[:], pattern=[[1, C]], base=0,
                           channel_multiplier=0)
            iota_f = constp.tile([128, C], f32, tag="iota_f")
            nc.vector.tensor_copy(iota_f[:], iota_i[:])

            # hoisted loads, biggest-latency-win order; wide tiles so each
            # is one DMA on the FIFO HWDGE ring
            qT = constp.tile([D, BL], f32, tag="qT")
            nc.sync.dma_start(out=qT[:], in_=qT_h[:])
            protoT = constp.tile([D, C], f32, tag="protoT")
            nc.sync.dma_start(out=protoT[:], in_=protoT_h[:])

            cls_all = constp.tile([128, 2 * C], f32, tag="cls01")
            cls_all2 = constp.tile([128, 2 * C], f32, tag="cls23")
            plab_all = constp.tile([128, 2 * C], f32, tag="plab01")
            plab_all2 = constp.tile([128, 2 * C], f32, tag="plab23")
            for half, (ct, pt) in enumerate(((cls_all, plab_all),
                                             (cls_all2, plab_all2))):
                rs = slice(half * 256, (half + 1) * 256)
                nc.sync.dma_start(
                    out=ct[:].rearrange("p (t c) -> p t c", t=2),
                    in_=cls_h[rs, :].rearrange("(t p) c -> p t c", p=128))
                nc.sync.dma_start(
                    out=pt[:].rearrange("p (t c) -> p t c", t=2),
                    in_=plab_h[rs, :].rearrange("(t p) c -> p t c", p=128))

            def cls_tile(t):
                buf = cls_all if t < 2 else cls_all2
                return buf[:, (t % 2) * C:(t % 2 + 1) * C]

            def plab_tile(t):
                buf = plab_all if t < 2 else plab_all2
                return buf[:, (t % 2) * C:(t % 2 + 1) * C]

            q_all = constp.tile([128, BL], f32, tag="q_all")
            nc.sync.dma_start(
                out=q_all[:].rearrange("p (t d) -> p t d", t=T),
                in_=q_h[:].rearrange("(t p) d -> p t d", p=128))
            ones = constp.tile([128, 128], bf16, tag="ones")
            nc.sync.dma_start(out=ones[:], in_=ones_h[:])
            tri = constp.tile([128, 128], bf16, tag="tri")
            last_load = nc.sync.dma_start(out=tri[:], in_=tri_h[:])

            # wide output tiles (single store DMA each)
            cls_o_all = constp.tile([128, T * C], f32, tag="cls_o_all")
            clu_o_all = constp.tile([128, T * C], f32, tag="clu_o_all")
            labs_all = constp.tile([128, T], f32, tag="labs_all")
            srT_all = constp.tile([128, C], f32, tag="srT_all")

            onehot = []
            with tc.tile_pool(name="pclu", bufs=2, space="PSUM") as pclu:
                for t in range(T):
                    # cluster logits (only needs qT/protoT -> starts early)
                    rs = slice(t * 128, (t + 1) * 128)
                    clu_ps = []
                    for ci, (c0, c1) in enumerate(CHUNKS):
                        pl = pclu.tile([128, c1 - c0], f32, tag=f"clu{ci}")
                        nc.tensor.matmul(pl[:], lhsT=qT[:, rs],
                                         rhs=protoT[:, c0:c1],
                                         start=True, stop=True)
                        clu_ps.append(pl)

                    # classifier softmax (no max-subtract: logits are O(5))
                    e_cls = workp.tile([128, C], f32, tag="e_cls")
                    sum_c = colp.tile([128, 1], f32, tag="sum_c")
                    nc.scalar.activation(e_cls[:], cls_tile(t), Act.Exp,
                                         bias=0.0, scale=1.0,
                                         accum_out=sum_c[:])
                    # cluster exps right behind on ACT so a stalled output
                    # copy can never wedge them (ACT executes in order)
                    e_clu = workp.tile([128, C], f32, tag="e_clu")
                    s0 = colp.tile([128, 1], f32, tag="s0")
                    s1 = colp.tile([128, 1], f32, tag="s1")
                    for ci, (c0, c1) in enumerate(CHUNKS):
                        nc.scalar.activation(e_clu[:, c0:c1], clu_ps[ci][:],
                                             Act.Exp, bias=0.0, scale=1.0,
                                             accum_out=(s0 if ci == 0 else s1)[:])

                    # argmax path: recip cancels inside a row; skip it here
                    masked = workp.tile([128, C], f32, tag="masked")
                    nc.vector.tensor_tensor(masked[:], e_cls[:], plab_tile(t),
                                            op=Alu.mult)
                    mx8 = colp.tile([128, 8], f32, tag="mx8")
                    nc.vector.max(mx8[:], masked[:])
                    ix8 = colp.tile([128, 8], u32, tag="ix8")
                    nc.vector.max_index(ix8[:], mx8[:], masked[:])
                    nc.vector.tensor_copy(labs_all[:, t:t + 1], ix8[:, 0:1])
                    oh = ohp.tile([128, C], bf16, tag="onehot", bufs=T,
                                  name=f"onehot{t}")
                    nc.vector.tensor_scalar(oh[:], iota_f[:],
                                            labs_all[:, t:t + 1], None,
                                            op0=Alu.is_equal)
                    onehot.append(oh)

                    # softmax outputs (leaves) into the wide tiles
                    rec_c = colp.tile([128, 1], f32, tag="rec_c")
                    nc.vector.reciprocal(rec_c[:], sum_c[:])
                    nc.scalar.activation(cls_o_all[:, t * C:(t + 1) * C],
                                         e_cls[:], Act.Copy,
                                         bias=0.0, scale=rec_c[:])
                    ssum = colp.tile([128, 1], f32, tag="ssum")
                    nc.vector.tensor_tensor(ssum[:], s0[:], s1[:], op=Alu.add)
                    rec_u = colp.tile([128, 1], f32, tag="rec_u")
                    nc.vector.reciprocal(rec_u[:], ssum[:])
                    nc.scalar.activation(clu_o_all[:, t * C:(t + 1) * C],
                                         e_clu[:], Act.Copy,
                                         bias=0.0, scale=rec_u[:])

                    # one bulk DRAM->DRAM queue chunk per tile iteration;
                    # gated behind the hoisted loads so the copies don't
                    # starve the latency-critical reads of HBM bandwidth
                    rows = QB // 4
                    d2d = nc.gpsimd.dma_start(
                        out=quc_h[t * rows:(t + 1) * rows, :],
                        in_=queue_h[t * rows:(t + 1) * rows, :])
                    add_dep_helper(d2d.ins, last_load.ins, sync=True,
                                   reason="bulk copy after critical loads")

            # remaining bulk passthrough copies
            for dst, src in ((qc_h, q_h), (kc_h, k_h), (qpc_h, qp_h)):
                d2d = nc.gpsimd.dma_start(out=dst[:], in_=src[:])
                add_dep_helper(d2d.ins, last_load.ins, sync=True,
                               reason="bulk copy after critical loads")

            # batched stores for the per-tile outputs
            nc.sync.dma_start(
                out=lab_h[:].rearrange("(t p) one -> p t one", p=128),
                in_=labs_all[:].rearrange("p (t one) -> p t one", t=T))
            nc.sync.dma_start(
                out=clso_h[:].rearrange("(t p) c -> p t c", p=128),
                in_=cls_o_all[:].rearrange("p (t c) -> p t c", t=T))
            nc.sync.dma_start(
                out=cluo_h[:].rearrange("(t p) c -> p t c", p=128),
                in_=clu_o_all[:].rearrange("p (t c) -> p t c", t=T))

            # ---------- suffix ranks + EMA weights + scatter ----------
            ow = []
            with (
                tc.tile_pool(name="pss", bufs=2, space="PSUM") as pss,
                tc.tile_pool(name="psc", bufs=1, space="PSUM") as psc,
            ):
                for t in range(T):
                    sA = colp.tile([128, 1], f32, tag="sA")
                    sB = colp.tile([128, 1], f32, tag="sB")
                    ss_ps = []
                    for ci, (c0, c1) in enumerate(CHUNKS):
                        ps = pss.tile([128, c1 - c0], f32, tag=f"ss{ci}")
                        for tj in range(t, T):
                            nc.tensor.matmul(ps[:],
                                             lhsT=(tri if tj == t else ones)[:],
                                             rhs=onehot[tj][:, c0:c1],
                                             start=(tj == t), stop=(tj == T - 1))
                        scr = workp.tile([128, c1 - c0], f32, tag=f"ttr{ci}")
                        nc.vector.tensor_tensor(scr[:], ps[:],
                                                onehot[t][:, c0:c1],
                                                op=Alu.mult)
                        nc.vector.tensor_reduce((sA if ci == 0 else sB)[:],
                                                scr[:],
                                                axis=mybir.AxisListType.X,
                                                op=Alu.add)
                        ss_ps.append(ps)

                    if t == 0:
                        # per-class counts = SS[0,:] + onehot[0,:]
                        cnt_sb = constp.tile([1, C], f32, tag="cnt_sb")
                        for ci, (c0, c1) in enumerate(CHUNKS):
                            nc.vector.tensor_tensor(cnt_sb[0:1, c0:c1],
                                                    ss_ps[ci][0:1, :],
                                                    onehot[0][0:1, c0:c1],
                                                    op=Alu.add)
                        nc.sync.dma_start(out=cnt_h[:], in_=cnt_sb[:])

                    s_col = colp.tile([128, 1], f32, tag="s_col")
                    nc.vector.tensor_tensor(s_col[:], sA[:], sB[:], op=Alu.add)
                    wloc = colp.tile([128, 1], f32, tag="wloc", bufs=T,
                                     name=f"wloc{t}")
                    nc.scalar.activation(wloc[:], s_col[:], Act.Exp,
                                         bias=0.0, scale=LN_M)
                    o_w = ohp.tile([128, C], f32, tag="ow", bufs=T,
                                   name=f"ow{t}")
                    nc.scalar.activation(o_w[:], onehot[t][:], Act.Copy,
                                         bias=0.0, scale=wloc[:])
                    ow.append(o_w)

                # partial scatter, transposed: S_r^T = sum_t q[t]^T @ ow[t]
                for ci, (c0, c1) in enumerate(CHUNKS):
                    ps = psc.tile([128, c1 - c0], f32, tag=f"sc{ci}")
                    for t in range(T):
                        nc.tensor.matmul(ps[:],
                                         lhsT=q_all[:, t * 128:(t + 1) * 128],
                                         rhs=ow[t][:, c0:c1],
                                         start=(t == 0), stop=(t == T - 1))
                    nc.scalar.copy(srT_all[:, c0:c1], ps[:])
                nc.sync.dma_start(out=srT_h[:], in_=srT_all[:])

    nc.compile()
    return nc


def _get_nc():
    if "nc" not in _CACHE:
        _CACHE["nc"] = _build()
    return _CACHE["nc"]


def kernel(q, k, classfy_logits, plabel, prototypes, queue, queue_pseudo, ptr):
    import ml_dtypes
    from concourse.bass_utils import run_bass_kernel_spmd

    q = np.ascontiguousarray(np.asarray(q, np.float32))
    k = np.ascontiguousarray(np.asarray(k, np.float32))
    cls = np.ascontiguousarray(np.asarray(classfy_logits, np.float32))
    plab = np.ascontiguousarray(np.asarray(plabel, np.float32))
    proto = np.ascontiguousarray(np.asarray(prototypes, np.float32))
    queue = np.ascontiguousarray(np.asarray(queue, np.float32))
    qp = np.ascontiguousarray(np.asarray(queue_pseudo, np.float32))
    ptr = int(np.asarray(ptr))

    protoT = np.ascontiguousarray(proto.T)
    ones128 = np.ones((128, 128), ml_dtypes.bfloat16)
    tri128 = np.tril(np.ones((128, 128), np.float32), -1).astype(ml_dtypes.bfloat16)

    in_maps = []
    for r in range(N_CORES):
        bs = slice(r * BL, (r + 1) * BL)
        qs = slice(r * QB, (r + 1) * QB)
        in_maps.append({
            "cls": cls[bs], "plab": plab[bs],
            "q": q[bs], "qT": np.ascontiguousarray(q[bs].T), "k": k[bs],
            "protoT": protoT, "queue_blk": queue[qs], "qp_blk": qp[qs],
            "ones128": ones128, "tri128": tri128,
        })

    nc = _get_nc()
    res = run_bass_kernel_spmd(nc, in_maps, list(range(N_CORES)))
    _CACHE["last_result"] = res
    out = res.results

    # ---------- host: gather / combine ----------
    m = np.float32(M_EMA)
    one_minus_m = np.float32(1.0 - M_EMA)

    cls_out = np.concatenate([out[r]["cls_out"] for r in range(N_CORES)], 0)
    clu_out = np.concatenate([out[r]["clu_out"] for r in range(N_CORES)], 0)
    labels = np.concatenate([out[r]["labels"][:, 0] for r in range(N_CORES)], 0)
    q_copies = [out[r]["q_copy"] for r in range(N_CORES)]
    k_copies = [out[r]["k_copy"] for r in range(N_CORES)]
    qu_copies = [out[r]["queue_copy"] for r in range(N_CORES)]
    qp_copies = [out[r]["qp_copy"] for r in range(N_CORES)]

    cont_features = np.concatenate(q_copies + k_copies + qu_copies, 0)
    qp_dev = np.concatenate(qp_copies, 0)
    cont_labels = np.concatenate([labels, labels, qp_dev], 0)

    p0 = min(max(ptr, 0), Q - B)   # jax dynamic_update_slice clamp
    new_queue = np.concatenate(qu_copies, 0)
    new_queue[p0:p0 + B] = np.concatenate(k_copies, 0)
    new_queue_pseudo = qp_dev.copy()
    new_queue_pseudo[p0:p0 + B] = labels

    # prototype scatter-EMA segment combine (tiny)
    counts = np.stack([out[r]["counts"][0] for r in range(N_CORES)], 0)  # [8,C]
    S_all = np.stack([np.ascontiguousarray(out[r]["s_rT"].T)
                      for r in range(N_CORES)], 0)                        # [8,C,D]
    suf = np.zeros((N_CORES, C), np.float32)
    acc = np.zeros((C,), np.float32)
    for r in range(N_CORES - 1, -1, -1):
        suf[r] = acc
        acc = acc + counts[r]
    scale = np.power(m, suf).astype(np.float32)
    scatter = np.einsum("rc,rcd->cd", scale, S_all).astype(np.float32)
    decay = np.power(m, counts.sum(0)).astype(np.float32)
    pre = proto * decay[:, None] + one_minus_m * scatter
    nrm = np.sqrt((pre * pre).sum(1, keepdims=True))
    new_prototypes = (pre / np.maximum(nrm, np.float32(1e-12))).astype(np.float32)

    new_ptr = (ptr + B) % Q
    return (cls_out, clu_out, cont_features, cont_labels,
            new_prototypes, new_queue, new_queue_pseudo, new_ptr)


# revision 15
# speedup vs baseline: 1.7101x; 1.2097x over previous
"""PiCO-style fused kernel for 8 Trainium2 NeuronCores (Bass/Tile).

Strategy (hardcoded from the sharding hint):
  - Data-parallel over the batch dim (4096 -> 512 rows/core) for the two
    softmaxes, the cluster matmul, the argmax pseudo-labels, and the
    per-core partial prototype scatter.
  - Prototypes replicated; the order-dependent scatter-EMA is decomposed
    per core (local suffix ranks via a triangular matmul, exact in bf16);
    the tiny [8,1000]+[8,1000,128] cross-core segment combine runs on
    the host.
  - Queue sharded by row (8192 rows/core), copied device-side
    (DRAM->DRAM); host only re-assembles full outputs from shards.
  - Batch-tile loads/stores are fused into single wide-tile DMAs (the
    HWDGE ring is FIFO; per-DMA fixed cost dominates at 0.5MB).
"""

import numpy as np

B, D, C, Q = 4096, 128, 1000, 65536
N_CORES = 8
BL = B // N_CORES          # 512 batch rows / core
QB = Q // N_CORES          # 8192 queue rows / core
T = BL // 128              # 4 partition tiles / core
M_EMA = 0.99
LN_M = float(np.log(0.99))
CHUNKS = ((0, 512), (512, C))   # psum-bank-sized column chunks of C

_CACHE: dict = {}


def _build():
    import concourse.bacc as bacc
    import concourse.tile as tile
    import concourse.mybir as mybir
    from concourse.tile import add_dep_helper

    f32 = mybir.dt.float32
    bf16 = mybir.dt.bfloat16
    i32 = mybir.dt.int32
    u32 = mybir.dt.uint32
    Alu = mybir.AluOpType
    Act = mybir.ActivationFunctionType

    nc = bacc.Bacc("TRN2", target_bir_lowering=False, debug=False,
                   num_devices=N_CORES)

    # ---- external I/O (per-core shapes) ----
    cls_h = nc.dram_tensor("cls", [BL, C], f32, kind="ExternalInput")
    plab_h = nc.dram_tensor("plab", [BL, C], f32, kind="ExternalInput")
    q_h = nc.dram_tensor("q", [BL, D], f32, kind="ExternalInput")
    qT_h = nc.dram_tensor("qT", [D, BL], f32, kind="ExternalInput")
    k_h = nc.dram_tensor("k", [BL, D], f32, kind="ExternalInput")
    protoT_h = nc.dram_tensor("protoT", [D, C], f32, kind="ExternalInput")
    queue_h = nc.dram_tensor("queue_blk", [QB, D], f32, kind="ExternalInput")
    qp_h = nc.dram_tensor("qp_blk", [QB], f32, kind="ExternalInput")
    ones_h = nc.dram_tensor("ones128", [128, 128], bf16, kind="ExternalInput")
    tri_h = nc.dram_tensor("tri128", [128, 128], bf16, kind="ExternalInput")

    clso_h = nc.dram_tensor("cls_out", [BL, C], f32, kind="ExternalOutput")
    cluo_h = nc.dram_tensor("clu_out", [BL, C], f32, kind="ExternalOutput")
    lab_h = nc.dram_tensor("labels", [BL, 1], f32, kind="ExternalOutput")
    cnt_h = nc.dram_tensor("counts", [1, C], f32, kind="ExternalOutput")
    srT_h = nc.dram_tensor("s_rT", [D, C], f32, kind="ExternalOutput")
    qc_h = nc.dram_tensor("q_copy", [BL, D], f32, kind="ExternalOutput")
    kc_h = nc.dram_tensor("k_copy", [BL, D], f32, kind="ExternalOutput")
    quc_h = nc.dram_tensor("queue_copy", [QB, D], f32, kind="ExternalOutput")
    qpc_h = nc.dram_tensor("qp_copy", [QB], f32, kind="ExternalOutput")

    with tile.TileContext(nc) as tc:
        with (
            tc.tile_pool(name="const", bufs=1) as constp,
            tc.tile_pool(name="work", bufs=3) as workp,
            tc.tile_pool(name="oh", bufs=4) as ohp,
            tc.tile_pool(name="cols", bufs=8) as colp,
        ):
            # iota first: sole gpsimd dependency of the onehot chain
            iota_i = constp.tile([128, C], i32, tag="iota_i")
            nc.gpsimd.iota(iota_i[:], pattern=[[1, C]], base=0,
                           channel_multiplier=0)
            iota_f = constp.tile([128, C], f32, tag="iota_f")
            nc.vector.tensor_copy(iota_f[:], iota_i[:])

            # hoisted loads, biggest-latency-win order; wide tiles so each
            # is one DMA on the FIFO HWDGE ring
            qT = constp.tile([D, BL], f32, tag="qT")
            nc.sync.dma_start(out=qT[:], in_=qT_h[:])
            protoT = constp.tile([D, C], f32, tag="protoT")
            nc.sync.dma_start(out=protoT[:], in_=protoT_h[:])
            ones = constp.tile([128, 128], bf16, tag="ones")
            nc.sync.dma_start(out=ones[:], in_=ones_h[:])
            tri = constp.tile([128, 128], bf16, tag="tri")
            nc.sync.dma_start(out=tri[:], in_=tri_h[:])

            cls_all = constp.tile([128, 2 * C], f32, tag="cls01")
            cls_all2 = constp.tile([128, 2 * C], f32, tag="cls23")
            plab_all = constp.tile([128, 2 * C], f32, tag="plab01")
            plab_all2 = constp.tile([128, 2 * C], f32, tag="plab23")
            d2d_gate = None
            for half, (ct, pt) in enumerate(((cls_all, plab_all),
                                             (cls_all2, plab_all2))):
                rs = slice(half * 256, (half + 1) * 256)
                nc.sync.dma_start(
                    out=ct[:].rearrange("p (t c) -> p t c", t=2),
                    in_=cls_h[rs, :].rearrange("(t p) c -> p t c", p=128))
                ld = nc.sync.dma_start(
                    out=pt[:].rearrange("p (t c) -> p t c", t=2),
                    in_=plab_h[rs, :].rearrange("(t p) c -> p t c", p=128))
                if half == 0:
                    d2d_gate = ld

            def cls_tile(t):
                buf = cls_all if t < 2 else cls_all2
                return buf[:, (t % 2) * C:(t % 2 + 1) * C]

            def plab_tile(t):
                buf = plab_all if t < 2 else plab_all2
                return buf[:, (t % 2) * C:(t % 2 + 1) * C]

            q_all = constp.tile([128, BL], f32, tag="q_all")
            nc.sync.dma_start(
                out=q_all[:].rearrange("p (t d) -> p t d", t=T),
                in_=q_h[:].rearrange("(t p) d -> p t d", p=128))

            # wide output tiles (single store DMA each)
            cls_o_all = constp.tile([128, T * C], f32, tag="cls_o_all")
            clu_o_all = constp.tile([128, T * C], f32, tag="clu_o_all")
            labs_all = constp.tile([128, T], f32, tag="labs_all")
            srT_all = constp.tile([128, C], f32, tag="srT_all")

            onehot = []
            with (
                tc.tile_pool(name="pclu", bufs=1, space="PSUM") as pclu,
                tc.tile_pool(name="pss", bufs=2, space="PSUM") as pss,
                tc.tile_pool(name="psc", bufs=1, space="PSUM") as psc,
            ):
                for t in range(T):
                    # cluster logits (only needs qT/protoT -> starts early)
                    rs = slice(t * 128, (t + 1) * 128)
                    clu_ps = []
                    for ci, (c0, c1) in enumerate(CHUNKS):
                        pl = pclu.tile([128, c1 - c0], f32, tag=f"clu{ci}")
                        nc.tensor.matmul(pl[:], lhsT=qT[:, rs],
                                         rhs=protoT[:, c0:c1],
                                         start=True, stop=True)
                        clu_ps.append(pl)

                    # classifier softmax (no max-subtract: logits are O(5))
                    e_cls = workp.tile([128, C], f32, tag="e_cls")
                    sum_c = colp.tile([128, 1], f32, tag="sum_c")
                    nc.scalar.activation(e_cls[:], cls_tile(t), Act.Exp,
                                         bias=0.0, scale=1.0,
                                         accum_out=sum_c[:])
                    # cluster exps right behind on ACT so a stalled output
                    # copy can never wedge them (ACT executes in order)
                    e_clu = workp.tile([128, C], f32, tag="e_clu")
                    s0 = colp.tile([128, 1], f32, tag="s0")
                    s1 = colp.tile([128, 1], f32, tag="s1")
                    for ci, (c0, c1) in enumerate(CHUNKS):
                        nc.scalar.activation(e_clu[:, c0:c1], clu_ps[ci][:],
                                             Act.Exp, bias=0.0, scale=1.0,
                                             accum_out=(s0 if ci == 0 else s1)[:])

                    # argmax path: recip cancels inside a row; skip it here
                    masked = workp.tile([128, C], f32, tag="masked")
                    nc.vector.tensor_tensor(masked[:], e_cls[:], plab_tile(t),
                                            op=Alu.mult)
                    mx8 = colp.tile([128, 8], f32, tag="mx8")
                    nc.vector.max(mx8[:], masked[:])
                    ix8 = colp.tile([128, 8], u32, tag="ix8")
                    nc.vector.max_index(ix8[:], mx8[:], masked[:])
                    nc.vector.tensor_copy(labs_all[:, t:t + 1], ix8[:, 0:1])
                    oh = ohp.tile([128, C], bf16, tag="onehot", bufs=T,
                                  name=f"onehot{t}")
                    nc.vector.tensor_scalar(oh[:], iota_f[:],
                                            labs_all[:, t:t + 1], None,
                                            op0=Alu.is_equal)
                    onehot.append(oh)

                    # softmax outputs (leaves) into the wide tiles
                    rec_c = colp.tile([128, 1], f32, tag="rec_c")
                    nc.vector.reciprocal(rec_c[:], sum_c[:])
                    nc.scalar.activation(cls_o_all[:, t * C:(t + 1) * C],
                                         e_cls[:], Act.Copy,
                                         bias=0.0, scale=rec_c[:])
                    ssum = colp.tile([128, 1], f32, tag="ssum")
                    nc.vector.tensor_tensor(ssum[:], s0[:], s1[:], op=Alu.add)
                    rec_u = colp.tile([128, 1], f32, tag="rec_u")
                    nc.vector.reciprocal(rec_u[:], ssum[:])
                    nc.scalar.activation(clu_o_all[:, t * C:(t + 1) * C],
                                         e_clu[:], Act.Copy,
                                         bias=0.0, scale=rec_u[:])

                    # one bulk DRAM->DRAM queue chunk per tile iteration;
                    # gated behind the hoisted loads so the copies don't
                    # starve the latency-critical reads of HBM bandwidth
                    rows = QB // 4
                    d2d = nc.gpsimd.dma_start(
                        out=quc_h[t * rows:(t + 1) * rows, :],
                        in_=queue_h[t * rows:(t + 1) * rows, :])
                    add_dep_helper(d2d.ins, d2d_gate.ins, sync=True,
                                   reason="bulk copy after critical loads")

                # remaining bulk passthrough copies
                for dst, src in ((qc_h, q_h), (kc_h, k_h), (qpc_h, qp_h)):
                    d2d = nc.gpsimd.dma_start(out=dst[:], in_=src[:])
                    add_dep_helper(d2d.ins, d2d_gate.ins, sync=True,
                                   reason="bulk copy after critical loads")

                # batched stores for the per-tile outputs
                nc.sync.dma_start(
                    out=lab_h[:].rearrange("(t p) one -> p t one", p=128),
                    in_=labs_all[:].rearrange("p (t one) -> p t one", t=T))
                nc.sync.dma_start(
                    out=clso_h[:].rearrange("(t p) c -> p t c", p=128),
                    in_=cls_o_all[:].rearrange("p (t c) -> p t c", t=T))
                nc.sync.dma_start(
                    out=cluo_h[:].rearrange("(t p) c -> p t c", p=128),
                    in_=clu_o_all[:].rearrange("p (t c) -> p t c", t=T))

                # ---------- suffix ranks + EMA weights + scatter ----------
                ow = []
                for t in range(T):
                    sA = colp.tile([128, 1], f32, tag="sA")
                    sB = colp.tile([128, 1], f32, tag="sB")
                    ss_ps = []
                    for ci, (c0, c1) in enumerate(CHUNKS):
                        ps = pss.tile([128, c1 - c0], f32, tag=f"ss{ci}")
                        for tj in range(t, T):
                            nc.tensor.matmul(ps[:],
                                             lhsT=(tri if tj == t else ones)[:],
                                             rhs=onehot[tj][:, c0:c1],
                                             start=(tj == t), stop=(tj == T - 1))
                        scr = workp.tile([128, c1 - c0], f32, tag=f"ttr{ci}")
                        nc.vector.tensor_tensor(scr[:], ps[:],
                                                onehot[t][:, c0:c1],
                                                op=Alu.mult)
                        nc.vector.tensor_reduce((sA if ci == 0 else sB)[:],
                                                scr[:],
                                                axis=mybir.AxisListType.X,
                                                op=Alu.add)
                        ss_ps.append(ps)

                    if t == 0:
                        # per-class counts = SS[0,:] + onehot[0,:]
                        cnt_sb = constp.tile([1, C], f32, tag="cnt_sb")
                        for ci, (c0, c1) in enumerate(CHUNKS):
                            nc.vector.tensor_tensor(cnt_sb[0:1, c0:c1],
                                                    ss_ps[ci][0:1, :],
                                                    onehot[0][0:1, c0:c1],
                                                    op=Alu.add)
                        nc.sync.dma_start(out=cnt_h[:], in_=cnt_sb[:])

                    s_col = colp.tile([128, 1], f32, tag="s_col")
                    nc.vector.tensor_tensor(s_col[:], sA[:], sB[:], op=Alu.add)
                    wloc = colp.tile([128, 1], f32, tag="wloc", bufs=T,
                                     name=f"wloc{t}")
                    nc.scalar.activation(wloc[:], s_col[:], Act.Exp,
                                         bias=0.0, scale=LN_M)
                    o_w = ohp.tile([128, C], f32, tag="ow", bufs=T,
                                   name=f"ow{t}")
                    nc.scalar.activation(o_w[:], onehot[t][:], Act.Copy,
                                         bias=0.0, scale=wloc[:])
                    ow.append(o_w)

                # partial scatter, transposed: S_r^T = sum_t q[t]^T @ ow[t]
                for ci, (c0, c1) in enumerate(CHUNKS):
                    ps = psc.tile([128, c1 - c0], f32, tag=f"sc{ci}")
                    for t in range(T):
                        nc.tensor.matmul(ps[:],
                                         lhsT=q_all[:, t * 128:(t + 1) * 128],
                                         rhs=ow[t][:, c0:c1],
                                         start=(t == 0), stop=(t == T - 1))
                    nc.scalar.copy(srT_all[:, c0:c1], ps[:])
                nc.sync.dma_start(out=srT_h[:], in_=srT_all[:])

    nc.compile()
    return nc


def _get_nc():
    if "nc" not in _CACHE:
        _CACHE["nc"] = _build()
    return _CACHE["nc"]


def kernel(q, k, classfy_logits, plabel, prototypes, queue, queue_pseudo, ptr):
    import ml_dtypes
    from concourse.bass_utils import run_bass_kernel_spmd

    q = np.ascontiguousarray(np.asarray(q, np.float32))
    k = np.ascontiguousarray(np.asarray(k, np.float32))
    cls = np.ascontiguousarray(np.asarray(classfy_logits, np.float32))
    plab = np.ascontiguousarray(np.asarray(plabel, np.float32))
    proto = np.ascontiguousarray(np.asarray(prototypes, np.float32))
    queue = np.ascontiguousarray(np.asarray(queue, np.float32))
    qp = np.ascontiguousarray(np.asarray(queue_pseudo, np.float32))
    ptr = int(np.asarray(ptr))

    protoT = np.ascontiguousarray(proto.T)
    ones128 = np.ones((128, 128), ml_dtypes.bfloat16)
    tri128 = np.tril(np.ones((128, 128), np.float32), -1).astype(ml_dtypes.bfloat16)

    in_maps = []
    for r in range(N_CORES):
        bs = slice(r * BL, (r + 1) * BL)
        qs = slice(r * QB, (r + 1) * QB)
        in_maps.append({
            "cls": cls[bs], "plab": plab[bs],
            "q": q[bs], "qT": np.ascontiguousarray(q[bs].T), "k": k[bs],
            "protoT": protoT, "queue_blk": queue[qs], "qp_blk": qp[qs],
            "ones128": ones128, "tri128": tri128,
        })

    nc = _get_nc()
    res = run_bass_kernel_spmd(nc, in_maps, list(range(N_CORES)))
    _CACHE["last_result"] = res
    out = res.results

    # ---------- host: gather / combine ----------
    m = np.float32(M_EMA)
    one_minus_m = np.float32(1.0 - M_EMA)

    cls_out = np.concatenate([out[r]["cls_out"] for r in range(N_CORES)], 0)
    clu_out = np.concatenate([out[r]["clu_out"] for r in range(N_CORES)], 0)
    labels = np.concatenate([out[r]["labels"][:, 0] for r in range(N_CORES)], 0)
    q_copies = [out[r]["q_copy"] for r in range(N_CORES)]
    k_copies = [out[r]["k_copy"] for r in range(N_CORES)]
    qu_copies = [out[r]["queue_copy"] for r in range(N_CORES)]
    qp_copies = [out[r]["qp_copy"] for r in range(N_CORES)]

    cont_features = np.concatenate(q_copies + k_copies + qu_copies, 0)
    qp_dev = np.concatenate(qp_copies, 0)
    cont_labels = np.concatenate([labels, labels, qp_dev], 0)

    p0 = min(max(ptr, 0), Q - B)   # jax dynamic_update_slice clamp
    new_queue = np.concatenate(qu_copies, 0)
    new_queue[p0:p0 + B] = np.concatenate(k_copies, 0)
    new_queue_pseudo = qp_dev.copy()
    new_queue_pseudo[p0:p0 + B] = labels

    # prototype scatter-EMA segment combine (tiny)
    counts = np.stack([out[r]["counts"][0] for r in range(N_CORES)], 0)  # [8,C]
    S_all = np.stack([np.ascontiguousarray(out[r]["s_rT"].T)
                      for r in range(N_CORES)], 0)                        # [8,C,D]
    suf = np.zeros((N_CORES, C), np.float32)
    acc = np.zeros((C,), np.float32)
    for r in range(N_CORES - 1, -1, -1):
        suf[r] = acc
        acc = acc + counts[r]
    scale = np.power(m, suf).astype(np.float32)
    scatter = np.einsum("rc,rcd->cd", scale, S_all).astype(np.float32)
    decay = np.power(m, counts.sum(0)).astype(np.float32)
    pre = proto * decay[:, None] + one_minus_m * scatter
    nrm = np.sqrt((pre * pre).sum(1, keepdims=True))
    new_prototypes = (pre / np.maximum(nrm, np.float32(1e-12))).astype(np.float32)

    new_ptr = (ptr + B) % Q
    return (cls_out, clu_out, cont_features, cont_labels,
            new_prototypes, new_queue, new_queue_pseudo, new_ptr)
